# revision 1
# baseline (speedup 1.0000x reference)
"""Trainium2 Bass kernel for DONNSentimentClassifier.

8-way batch-parallel (32 batch rows per core). Per core:
  x -> gather from host-precomputed tables Tc = DT^1.5*SCALE*relu(E@W1c + b1c)
    -> Hopf layer 1: sqrt(DT)-scaled Euler recurrence, 7 stock DVE instrs/substep
    -> h1/G2 matmuls (PE+ACT) -> Hopf layer 2 -> h2/tanh/logits (PE+ACT) -> out

Layout: 64 partitions = oscillator u; components (wr, wi) are column halves, so
every scalar_tensor_tensor reads same-base-partition operands (HW constraint).
Euler substep with w = sqrt(DT)*z, c0 = 1+DT*MU, beta = DT*omega:
  S = Z*Z; A' = -S_r - S_i; P_c = (A'+c0)*Z_c; U_r = -beta*Z_i + G_r;
  U_i = beta*Z_r + G_i; Z' = P + U
"""

import sys

for p in ("/opt/trn_rl_repo", "/root/.axon_site/_ro/trn_rl_repo"):
    if p not in sys.path:
        sys.path.append(p)

import numpy as np

import concourse.bass as bass
import concourse.mybir as mybir
from concourse.bass_utils import run_bass_kernel_spmd
from concourse.tile import TileContext

F32 = mybir.dt.float32
AF = mybir.ActivationFunctionType
OP = mybir.AluOpType

B, T, U, ED, V, PD, NC_OUT = 256, 512, 64, 100, 32000, 20, 2
NUM_STEPS = 20
DT = np.float32(1e-3)
SCALE = np.float32(0.2)
MU = np.float32(1.0)
SQDT = np.sqrt(DT).astype(np.float32)
C0 = float(np.float32(1.0) + DT * MU)
N_CORES = 8
BS = B // N_CORES  # 32 batch rows per core
NTOK = T * BS  # 16384 token-batch columns per core

_CACHE = {}


_ENG_SEM = {
    "DVE": "DVE_", "Activation": "Activation_", "PE": "PE_",
    "Pool": "Pool_", "SP": "SP_",
}


def _is_self_wait(ins, w):
    """Wait on the instruction's own engine semaphore. NOTE: eliding these
    raced with the For_i back-edge semaphore reset (rel err 1.7e-5 -> 7e-3),
    so elision is disabled; kept for documentation."""
    return False
    pref = _ENG_SEM.get(getattr(ins.engine, "name", ""), None)
    n = getattr(w, "ant_name", "") or ""
    return (
        pref is not None
        and n.startswith(pref)
        and n[len(pref):].isdigit()
    )


def _split_waits(nc, cap=1):
    """This walrus build allows ~1 sync-wait per instruction; drop redundant
    same-engine waits, then move excess waits onto single-wait NoOps."""
    nop_id = [0]
    for f in nc.m.functions:
        for bb in f.blocks:
            il = bb.instructions
            pos = 0
            while pos < len(il):
                ins = il[pos]
                si = ins.sync_info
                if si is not None and si.on_wait:
                    kept = [w for w in si.on_wait if not _is_self_wait(ins, w)]
                    if len(kept) != len(si.on_wait):
                        ins.sync_info = mybir.SyncInfo(
                            on_wait=kept, on_update=list(si.on_update or [])
                        )
                        si = ins.sync_info
                if si is None or si.on_wait is None or len(si.on_wait) <= cap:
                    pos += 1
                    continue
                waits = list(si.on_wait)
                keep, extra = waits[-cap:], waits[:-cap]
                for w in extra:
                    nop = mybir.InstNoOp(
                        name=f"waitnop_{nop_id[0]}", ins=[], outs=[]
                    )
                    nop_id[0] += 1
                    nop.engine = ins.engine
                    nop.sync_info = mybir.SyncInfo(on_wait=[w], on_update=[])
                    il.insert(pos, nop)
                    pos += 1
                ins.sync_info = mybir.SyncInfo(
                    on_wait=keep, on_update=list(si.on_update or [])
                )
                pos += 1


def _build(debug_states=False):
    key = ("nc", debug_states)
    if key in _CACHE:
        return _CACHE[key]
    nc = bass.Bass()

    g1 = nc.declare_dram_parameter("g1", [U, 2 * NTOK], F32, isOutput=False)
    # wp1a/b: [U, U] halves of Wp1/sqrt(DT); w2r/w2i scaled; wp2a/b likewise
    wp1a = nc.declare_dram_parameter("wp1a", [U, U], F32, isOutput=False)
    wp1b = nc.declare_dram_parameter("wp1b", [U, U], F32, isOutput=False)
    bp1 = nc.declare_dram_parameter("bp1", [U, 1], F32, isOutput=False)
    w2r = nc.declare_dram_parameter("w2r", [U, U], F32, isOutput=False)
    w2i = nc.declare_dram_parameter("w2i", [U, U], F32, isOutput=False)
    b2r = nc.declare_dram_parameter("b2r", [U, 1], F32, isOutput=False)
    b2i = nc.declare_dram_parameter("b2i", [U, 1], F32, isOutput=False)
    wp2a = nc.declare_dram_parameter("wp2a", [U, U], F32, isOutput=False)
    wp2b = nc.declare_dram_parameter("wp2b", [U, U], F32, isOutput=False)
    bp2 = nc.declare_dram_parameter("bp2", [U, 1], F32, isOutput=False)
    wpr = nc.declare_dram_parameter("wpr", [U, PD], F32, isOutput=False)
    bpr = nc.declare_dram_parameter("bpr", [PD, 1], F32, isOutput=False)
    wh = nc.declare_dram_parameter("wh", [PD, NC_OUT], F32, isOutput=False)
    bh = nc.declare_dram_parameter("bh", [NC_OUT, 1], F32, isOutput=False)
    bpos = nc.declare_dram_parameter("bpos", [U, 1], F32, isOutput=False)
    bneg = nc.declare_dram_parameter("bneg", [U, 1], F32, isOutput=False)
    w0 = nc.declare_dram_parameter("w0", [U, 2 * BS], F32, isOutput=False)
    out = nc.declare_dram_parameter("out", [NC_OUT, NTOK], F32, isOutput=True)
    sdram = nc.dram_tensor("sdram", [U, 2 * NTOK], F32)
    g2d = nc.dram_tensor("g2d", [U, 2 * NTOK], F32)
    if debug_states:
        dbg1 = nc.declare_dram_parameter("dbg1", [U, 2 * NTOK], F32, isOutput=True)
        dbg2 = nc.declare_dram_parameter("dbg2", [U, 2 * NTOK], F32, isOutput=True)

    ident = None
    from contextlib import ExitStack
    with TileContext(nc) as tc, ExitStack() as _es:

        w0_t = _es.enter_context(nc.sbuf_tensor([U, 2 * BS], F32))
        bpos_t = _es.enter_context(nc.sbuf_tensor([U, 1], F32))
        bneg_t = _es.enter_context(nc.sbuf_tensor([U, 1], F32))
        wp1a_t = _es.enter_context(nc.sbuf_tensor([U, U], F32))
        wp1b_t = _es.enter_context(nc.sbuf_tensor([U, U], F32))
        bp1_t = _es.enter_context(nc.sbuf_tensor([U, 1], F32))
        w2r_t = _es.enter_context(nc.sbuf_tensor([U, U], F32))
        w2i_t = _es.enter_context(nc.sbuf_tensor([U, U], F32))
        b2r_t = _es.enter_context(nc.sbuf_tensor([U, 1], F32))
        b2i_t = _es.enter_context(nc.sbuf_tensor([U, 1], F32))
        wp2a_t = _es.enter_context(nc.sbuf_tensor([U, U], F32))
        wp2b_t = _es.enter_context(nc.sbuf_tensor([U, U], F32))
        bp2_t = _es.enter_context(nc.sbuf_tensor([U, 1], F32))
        wpr_t = _es.enter_context(nc.sbuf_tensor([U, PD], F32))
        bpr_t = _es.enter_context(nc.sbuf_tensor([PD, 1], F32))
        wh_t = _es.enter_context(nc.sbuf_tensor([PD, NC_OUT], F32))
        bh_t = _es.enter_context(nc.sbuf_tensor([NC_OUT, 1], F32))
        if True:
            for dst, src in (
                (bpos_t, bpos), (bneg_t, bneg), (w0_t, w0), (wp1a_t, wp1a),
                (wp1b_t, wp1b), (bp1_t, bp1), (w2r_t, w2r), (w2i_t, w2i),
                (b2r_t, b2r), (b2i_t, b2i), (wp2a_t, wp2a), (wp2b_t, wp2b),
                (bp2_t, bp2), (wpr_t, wpr), (bpr_t, bpr), (wh_t, wh), (bh_t, bh),
            ):
                nc.sync.dma_start(out=dst if isinstance(dst, bass.AP) else dst[:], in_=src[:])
            # warm weights through DVE so matmul operand deps share one semaphore
            for wt in (wp1a_t, wp1b_t, w2r_t, w2i_t, wp2a_t, wp2b_t, wpr_t, wh_t):
                a = wt if isinstance(wt, bass.AP) else wt[:]
                nc.vector.tensor_scalar_mul(out=a, in0=a, scalar1=1.0)


            def hopf_phase(pool, zst, gsrc):
                nc.vector.tensor_copy(out=zst[:], in_=w0_t[:])
                with tc.For_i(0, T, 1, name="hopf") as i:
                    gt = pool.tile([U, 2 * BS], F32, tag="gt")
                    nc.sync.dma_start(
                        out=gt[:], in_=gsrc[:, bass.ds(i * 2 * BS, 2 * BS)]
                    )
                    g_r = gt[:, 0:BS]
                    g_i = gt[:, BS:2 * BS]
                    cur = zst
                    for k in range(NUM_STEPS):
                        s = pool.tile([U, 2 * BS], F32, tag=f"s{k % 2}")
                        ap_ = pool.tile([U, BS], F32, tag=f"a{k % 2}")
                        p = pool.tile([U, 2 * BS], F32, tag=f"p{k % 2}")
                        uu = pool.tile([U, 2 * BS], F32, tag=f"u{k % 2}")
                        nc.vector.tensor_tensor(
                            out=s[:], in0=cur[:], in1=cur[:], op=OP.mult
                        )
                        nc.vector.scalar_tensor_tensor(
                            out=ap_[:], in0=s[:, 0:BS], scalar=-1.0,
                            in1=s[:, BS:2 * BS], op0=OP.mult, op1=OP.subtract,
                        )
                        nc.vector.scalar_tensor_tensor(
                            out=p[:], in0=ap_[:].rearrange('u (x b) -> u x b', x=1).to_broadcast([U, 2, BS]),
                            scalar=C0, in1=cur[:], op0=OP.add, op1=OP.mult,
                        )
                        nc.vector.scalar_tensor_tensor(
                            out=uu[:, 0:BS], in0=cur[:, BS:2 * BS],
                            scalar=bneg_t[:, :], in1=g_r,
                            op0=OP.mult, op1=OP.add,
                        )
                        nc.vector.scalar_tensor_tensor(
                            out=uu[:, BS:2 * BS], in0=cur[:, 0:BS],
                            scalar=bpos_t[:, :], in1=g_i,
                            op0=OP.mult, op1=OP.add,
                        )
                        nc.vector.tensor_tensor(
                            out=zst[:], in0=p[:], in1=uu[:], op=OP.add
                        )
                    nc.sync.dma_start(
                        out=sdram[:, bass.ds(i * 2 * BS, 2 * BS)], in_=zst[:]
                    )

            def load_states(pool, t0, NT):
                sv = pool.tile([U, NT * 2 * BS], F32, tag="sv")
                nc.sync.dma_start(
                    out=sv[:], in_=sdram[:, t0 * 2 * BS:(t0 + NT) * 2 * BS]
                )
                v = sv[:].rearrange("u (t c b) -> u t c b", t=NT, c=2, b=BS)
                return v[:, :, 0, :], v[:, :, 1, :]

            def mid_matmuls(pool, psum_pool):
                NT = 8  # tokens per chunk -> N = 256
                for cnk in range(T // NT):
                    t0 = cnk * NT
                    N = NT * BS
                    rv, iv = load_states(pool, t0, NT)
                    ph1 = psum_pool.tile([U, N], F32, tag="mm")
                    h1 = pool.tile([U, N], F32, tag="h1")
                    pg = psum_pool.tile([U, N], F32, tag="mm2")
                    nc.tensor.matmul(
                        out=ph1[:], lhsT=wp1a_t[:], rhs=rv,
                        start=True, stop=False,
                    )
                    nc.tensor.matmul(
                        out=ph1[:], lhsT=wp1b_t[:], rhs=iv,
                        start=False, stop=True,
                    )
                    nc.scalar.activation(
                        out=h1[:], in_=ph1[:], func=AF.Relu, bias=bp1_t[:, :]
                    )
                    g2t = pool.tile([U, NT * 2 * BS], F32, tag="g2t")
                    g2v = g2t[:].rearrange("u (t c b) -> u t c b", t=NT, c=2, b=BS)
                    for c, (wt, bt) in enumerate(
                        ((w2r_t, b2r_t), (w2i_t, b2i_t))
                    ):
                        nc.tensor.matmul(
                            out=pg[:], lhsT=wt[:], rhs=h1[:],
                            start=True, stop=True,
                        )
                        nc.scalar.activation(
                            out=g2v[:, :, c, :], in_=pg[:].rearrange(
                                "u (t b) -> u t b", t=NT, b=BS
                            ),
                            func=AF.Relu, bias=bt[:, :],
                        )
                    nc.sync.dma_start(
                        out=g2d[:, t0 * 2 * BS:(t0 + NT) * 2 * BS], in_=g2t[:]
                    )

            def final_matmuls(pool, psum_pool):
                NT = 8
                for cnk in range(T // NT):
                    t0 = cnk * NT
                    N = NT * BS
                    rv, iv = load_states(pool, t0, NT)
                    ph2 = psum_pool.tile([U, N], F32, tag="mm")
                    h2 = pool.tile([U, N], F32, tag="h1")
                    ph3 = psum_pool.tile([PD, N], F32, tag="mm2")
                    h3 = pool.tile([PD, N], F32, tag="h3")
                    pl = psum_pool.tile([NC_OUT, N], F32, tag="mm3")
                    lg = pool.tile([NC_OUT, N], F32, tag="lg")
                    nc.tensor.matmul(
                        out=ph2[:], lhsT=wp2a_t[:], rhs=rv,
                        start=True, stop=False,
                    )
                    nc.tensor.matmul(
                        out=ph2[:], lhsT=wp2b_t[:], rhs=iv,
                        start=False, stop=True,
                    )
                    nc.scalar.activation(
                        out=h2[:], in_=ph2[:], func=AF.Relu, bias=bp2_t[:, :]
                    )
                    nc.tensor.matmul(
                        out=ph3[:], lhsT=wpr_t[:], rhs=h2[:], start=True, stop=True
                    )
                    nc.scalar.activation(
                        out=h3[:], in_=ph3[:], func=AF.Tanh, bias=bpr_t[:, :]
                    )
                    nc.tensor.matmul(
                        out=pl[:], lhsT=wh_t[:], rhs=h3[:], start=True, stop=True
                    )
                    nc.scalar.activation(
                        out=lg[:], in_=pl[:], func=AF.Identity, bias=bh_t[:, :]
                    )
                    nc.sync.dma_start(
                        out=out[:, t0 * BS:(t0 + NT) * BS], in_=lg[:]
                    )

            if True:
                pool = _es.enter_context(tc.tile_pool(name="work", bufs=3))
                psum_pool = _es.enter_context(
                    tc.tile_pool(name="psum", bufs=2, space="PSUM")
                )
                zst = _es.enter_context(nc.sbuf_tensor([U, 2 * BS], F32))
                hopf_phase(pool, zst, g1)
                tc.strict_bb_all_engine_barrier()
                if debug_states:
                    nc.sync.dma_start(out=dbg1[:], in_=sdram[:])
                mid_matmuls(pool, psum_pool)
                tc.strict_bb_all_engine_barrier()
                hopf_phase(pool, zst, g2d)
                tc.strict_bb_all_engine_barrier()
                if debug_states:
                    nc.sync.dma_start(out=dbg2[:], in_=sdram[:])
                final_matmuls(pool, psum_pool)

    _split_waits(nc)
    _CACHE[key] = nc
    return nc


def _host_precompute(inp):
    f32 = np.float32
    GS = (DT * SQDT * SCALE).astype(f32)
    E = inp["E"]
    t1r = (GS * np.maximum(E @ inp["W1r"] + inp["b1r"], 0)).astype(f32)
    t1i = (GS * np.maximum(E @ inp["W1i"] + inp["b1i"], 0)).astype(f32)
    beta = (DT * inp["om1"]).astype(f32)
    beta2 = (DT * inp["om2"]).astype(f32)
    assert np.allclose(beta, beta2), "kernel assumes om1 == om2"
    w0 = np.zeros((U, 2 * BS), f32)
    w0[:, 0:BS] = (np.full((U, BS), f32(0.1)) * SQDT).astype(f32)
    wp1s = (inp["Wp1"] / SQDT).astype(f32)
    wp2s = (inp["Wp2"] / SQDT).astype(f32)
    return {
        "_t1r": np.ascontiguousarray(t1r),
        "_t1i": np.ascontiguousarray(t1i),
        "wp1a": np.ascontiguousarray(wp1s[:U]),
        "wp1b": np.ascontiguousarray(wp1s[U:]),
        "bp1": inp["bp1"][:, None].astype(f32),
        "w2r": (GS * inp["W2r"]).astype(f32),
        "w2i": (GS * inp["W2i"]).astype(f32),
        "b2r": (GS * inp["b2r"])[:, None].astype(f32),
        "b2i": (GS * inp["b2i"])[:, None].astype(f32),
        "wp2a": np.ascontiguousarray(wp2s[:U]),
        "wp2b": np.ascontiguousarray(wp2s[U:]),
        "bp2": inp["bp2"][:, None].astype(f32),
        "wpr": inp["Wpr"].astype(f32),
        "bpr": inp["bpr"][:, None].astype(f32),
        "wh": inp["Wh"].astype(f32),
        "bh": inp["bh"][:, None].astype(f32),
        "bpos": beta[:, None].astype(f32),
        "bneg": (-beta)[:, None].astype(f32),
        "w0": w0,
    }


def kernel(trace=False, debug_states=False, **inputs):
    x = np.asarray(inputs["x"]).astype(np.int32)
    inp = {k: np.asarray(v).astype(np.float32) for k, v in inputs.items() if k != "x"}
    common = _host_precompute(inp)
    t1r = common.pop("_t1r")
    t1i = common.pop("_t1i")
    nc = _build(debug_states=debug_states)
    in_maps = []
    for c in range(N_CORES):
        xs = x[c * BS:(c + 1) * BS]  # [BS, T]
        m = dict(common)
        # g1[u, (t, c, b)] = t1c[x[b, t], u]
        g = np.stack([t1r[xs], t1i[xs]], axis=0)  # [c, BS, T, U]
        m["g1"] = np.ascontiguousarray(
            g.transpose(3, 2, 0, 1).reshape(U, 2 * NTOK)
        )
        in_maps.append(m)
    res = run_bass_kernel_spmd(
        nc, in_maps, core_ids=list(range(N_CORES)), trace=trace
    )
    out = np.empty((B, T, NC_OUT), np.float32)
    for c in range(N_CORES):
        o = res.results[c]["out"].reshape(NC_OUT, T, BS)  # cols (t, b)
        out[c * BS:(c + 1) * BS] = o.transpose(2, 1, 0)
    if debug_states or trace:
        kernel.last_result = res
    return out

